# revision 3
# baseline (speedup 1.0000x reference)
"""Gather + segment-mean (GNN aggregation) Trainium2 kernel.

Problem: out[s] = mean over edges e with segment_ids[e]==s of values[gather_idx[e]]
  values      [50000, 128] f32
  gather_idx  [640000] i64
  segment_ids [640000] i64 (sorted)
  num_segments 40000
  out         [40000, 128] f32

Strategy (8 NeuronCores, SPMD, full inputs in / full output out):
  - Shard by output segments: 5000 segments per core (disjoint -> no
    cross-core reduce). Each core sees the full values table.
  - On each core, segments are LPT-packed into W=40 "windows" of <=128
    segments.  A window's edges are gathered with the Q7 dma_gather
    custom instruction (int16 indices -> rows land at partition i%128,
    chunk i//128), then reduced with PE matmuls: for each 128-edge chunk,
    S[e, j] = (seg_local[e] == j) built by a DVE is_equal against an iota
    row, and psum[j, :] += S.T @ gathered accumulates the per-window
    segment sums.  ACT divides by counts (per-partition scale) and the
    window's 128 rows stream back to DRAM.
  - dma_gather indices are int16 (< 32768), so the values table is split
    into a low half [0, 32768) and a high half [32768, 50000); each
    window keeps its low and high edges in separate chunk ranges and two
    gathers per batch fetch them.
  - Window packing is data-dependent, so the host computes it from the
    actual index arrays at call time; the Bass program itself is uniform
    across cores (SPMD) with shapes derived from the packing.

The final output permutation (window packing scrambles segment order) is
undone on the host while unsharding.
"""

import os
import sys

sys.path.insert(0, "/opt/trn_rl_repo")

import numpy as np

P = 128
D = 128
NCORES = 8
HALF = 32768
B = 8  # windows per gather batch

_COMPILED = {}  # (shape key) -> (nc, static config)


# --------------------------------------------------------------------------
# host-side packing
# --------------------------------------------------------------------------

def _pack_core(n_low, n_high, nwin):
    """LPT pack len(n_low) segments into nwin windows (<=128 segs each),
    balancing low/high edge loads. Returns assign [nseg] window ids."""
    nseg = len(n_low)
    tot = n_low + n_high
    order = np.argsort(-tot, kind="stable")
    win_low = np.zeros(nwin, np.float64)
    win_high = np.zeros(nwin, np.float64)
    win_nseg = np.zeros(nwin, np.int64)
    assign = np.empty(nseg, np.int64)
    # normalizers: expected per-window loads
    tl = max(n_low.sum() / nwin, 1.0)
    th = max(n_high.sum() / nwin, 1.0)
    for s in order:
        load = np.maximum((win_low + n_low[s]) / tl, (win_high + n_high[s]) / th)
        load[win_nseg >= P] = np.inf
        w = int(np.argmin(load))
        assign[s] = w
        win_low[w] += n_low[s]
        win_high[w] += n_high[s]
        win_nseg[w] += 1
    return assign


def _wrap_idx(idx):
    """[n] int16 -> [128, n//16] wrapped (i%16, i//16), replicated x8."""
    n = idx.shape[0]
    img16 = idx.reshape(n // 16, 16).T
    return np.tile(img16, (8, 1)).astype(np.int16)


def _build_host(values, gather_idx, segment_ids, num_segments):
    """Compute packing + per-core input arrays. Returns (cfg, in_maps, perm)."""
    n_src = values.shape[0]
    seg_per_core = num_segments // NCORES
    gather_idx = np.asarray(gather_idx).astype(np.int64)
    segment_ids = np.asarray(segment_ids).astype(np.int64)

    counts_all = np.bincount(segment_ids, minlength=num_segments)
    is_high = gather_idx >= HALF

    # per-core edge ranges (segment_ids sorted)
    core_edge_lo = np.searchsorted(segment_ids, np.arange(NCORES) * seg_per_core)
    core_edge_hi = np.searchsorted(
        segment_ids, (np.arange(NCORES) + 1) * seg_per_core
    )

    nwin = -(-seg_per_core // P)          # 40 for 5000
    nwin = -(-nwin // B) * B              # multiple of B so batches are full
    cores = []
    max_cl = 1
    max_ch = 1
    for c in range(NCORES):
        e0, e1 = core_edge_lo[c], core_edge_hi[c]
        seg_l = segment_ids[e0:e1] - c * seg_per_core  # local seg of each edge
        idx_c = gather_idx[e0:e1]
        hi_c = is_high[e0:e1]
        n_low = np.bincount(seg_l[~hi_c], minlength=seg_per_core)
        n_high = np.bincount(seg_l[hi_c], minlength=seg_per_core)
        assign = _pack_core(n_low, n_high, nwin)
        # j = rank of segment within its window (segment-sorted order)
        order = np.lexsort((np.arange(seg_per_core), assign))
        j_of = np.empty(seg_per_core, np.int64)
        pos_in_win = np.empty(seg_per_core, np.int64)
        # positions within each window
        w_sorted = assign[order]
        starts = np.searchsorted(w_sorted, np.arange(nwin))
        pos_in_win[order] = np.arange(seg_per_core) - starts[w_sorted]
        j_of = pos_in_win
        win_low = np.bincount(assign, weights=n_low, minlength=nwin).astype(np.int64)
        win_high = np.bincount(assign, weights=n_high, minlength=nwin).astype(np.int64)
        max_cl = max(max_cl, int(-(-win_low.max() // P)))
        max_ch = max(max_ch, int(-(-win_high.max() // P)))
        cores.append(
            dict(e0=e0, e1=e1, seg_l=seg_l, idx=idx_c, hi=hi_c,
                 assign=assign, j_of=j_of)
        )

    CL, CH = max_cl, max_ch
    nb = -(-nwin // B)
    nwin_pad = nb * B
    cpw = CL + CH  # chunks per window
    batch_chunks = B * cpw
    batch_slots = batch_chunks * P
    total_slots = nb * batch_slots

    cfg = dict(CL=CL, CH=CH, NB=nb, NWIN=nwin_pad, n_src=n_src,
               seg_per_core=seg_per_core, num_segments=num_segments)

    iota_np = np.tile(np.arange(P, dtype=np.float32), (P, 1))
    values_lo = np.ascontiguousarray(values[:HALF]).astype(np.float32, copy=False)
    values_hi = np.ascontiguousarray(values[HALF:]).astype(np.float32, copy=False)

    in_maps = []
    perm = np.empty(num_segments, np.int64)
    for c in range(NCORES):
        cc = cores[c]
        win = cc["assign"][cc["seg_l"]]  # per edge window
        j = cc["j_of"][cc["seg_l"]]      # per edge psum row
        hi = cc["hi"].astype(np.int64)
        # order edges by (window, region, segment)
        order = np.lexsort((j, hi, win))
        w_s, h_s = win[order], hi[order]
        grp = w_s * 2 + h_s
        # offset within (window, region) group
        uniq, grp_start = np.unique(grp, return_index=True)
        start_of = np.zeros(nwin_pad * 2, np.int64)
        start_of[uniq] = grp_start
        off = np.arange(len(order)) - start_of[grp]
        b_of = w_s // B
        wb = w_s % B
        base = b_of * batch_slots + np.where(
            h_s == 0, wb * CL * P, B * CL * P + wb * CH * P
        )
        slot = base + off
        idx_all = np.zeros(total_slots, np.int64)
        seg_all = np.full(total_slots, -1.0, np.float32)
        idx_sorted = cc["idx"][order]
        idx_all[slot] = np.where(h_s == 1, idx_sorted - HALF, idx_sorted)
        seg_all[slot] = j[order]

        # images
        idx_img = np.empty((nb, P, batch_chunks * 8), np.int16)
        seg_img = np.empty((nb, P, batch_chunks), np.float32)
        for b in range(nb):
            lo = idx_all[b * batch_slots : b * batch_slots + B * CL * P]
            hi_r = idx_all[b * batch_slots + B * CL * P : (b + 1) * batch_slots]
            idx_img[b, :, : B * CL * 8] = _wrap_idx(lo.astype(np.int16))
            idx_img[b, :, B * CL * 8 :] = _wrap_idx(hi_r.astype(np.int16))
            seg_img[b] = (
                seg_all[b * batch_slots : (b + 1) * batch_slots]
                .reshape(batch_chunks, P)
                .T
            )

        # recip counts per (window, j)
        rcp_img = np.ones((nb, P, B), np.float32)
        seg_global = np.arange(seg_per_core) + c * seg_per_core
        cnt = counts_all[seg_global].astype(np.float32)
        rcp = 1.0 / np.maximum(cnt, 1.0)
        aw, aj = cores[c]["assign"], cores[c]["j_of"]
        rcp_img[aw // B, aj, aw % B] = rcp
        perm[seg_global] = c * nwin_pad * P + aw * P + aj

        in_maps.append(
            {
                "values_lo": values_lo,
                "values_hi": values_hi,
                "idx_img": idx_img[c:c+1][0] if False else idx_img,
                "seg_img": seg_img,
                "rcp_img": rcp_img,
                "iota": iota_np,
            }
        )
    return cfg, in_maps, perm


# --------------------------------------------------------------------------
# bass program
# --------------------------------------------------------------------------

def _build_program(cfg):
    import concourse.bacc as bacc
    import concourse.mybir as mybir
    import concourse.tile as tile

    CL, CH, NB = cfg["CL"], cfg["CH"], cfg["NB"]
    n_src = cfg["n_src"]
    cpw = CL + CH
    batch_chunks = B * cpw
    nwin_pad = cfg["NWIN"]

    nc = bacc.Bacc("TRN2", target_bir_lowering=False)
    f32 = mybir.dt.float32
    i16 = mybir.dt.int16

    v_lo = nc.declare_dram_parameter("values_lo", [HALF, D], f32, isOutput=False)
    v_hi = nc.declare_dram_parameter("values_hi", [n_src - HALF, D], f32, isOutput=False)
    idx_t = nc.declare_dram_parameter(
        "idx_img", [NB, P, batch_chunks * 8], i16, isOutput=False
    )
    seg_t = nc.declare_dram_parameter(
        "seg_img", [NB, P, batch_chunks], f32, isOutput=False
    )
    rcp_t = nc.declare_dram_parameter("rcp_img", [NB, P, B], f32, isOutput=False)
    iota_t = nc.declare_dram_parameter("iota", [P, P], f32, isOutput=False)
    out_t = nc.declare_dram_parameter("out", [nwin_pad * P, D], f32, isOutput=True)

    with tile.TileContext(nc) as tc:
        with (
            tc.tile_pool(name="gb", bufs=2) as gb,
            tc.tile_pool(name="meta", bufs=2) as meta,
            tc.tile_pool(name="sp", bufs=6) as sp,
            tc.tile_pool(name="ob", bufs=4) as ob,
            tc.tile_pool(name="const", bufs=1) as cpool,
            tc.tile_pool(name="ps", bufs=4, space="PSUM") as ps,
        ):
            iota = cpool.tile([P, P], f32)
            nc.sync.dma_start(iota[:], iota_t[:])
            for b in range(NB):
                idxs = meta.tile([P, batch_chunks * 8], i16, tag="idx")
                nc.sync.dma_start(idxs[:], idx_t[b])
                segs = meta.tile([P, batch_chunks], f32, tag="seg")
                nc.sync.dma_start(segs[:], seg_t[b])
                rcp = meta.tile([P, B], f32, tag="rcp")
                nc.sync.dma_start(rcp[:], rcp_t[b])

                g = gb.tile([P, batch_chunks, D], f32, tag="g")
                nc.gpsimd.dma_gather(
                    out_ap=g[:, : B * CL, :],
                    in_ap=v_lo[:],
                    idxs_ap=idxs[:, : B * CL * 8],
                    num_idxs=B * CL * P,
                    num_idxs_reg=B * CL * P,
                    elem_size=D,
                    single_packet=False,
                )
                nc.gpsimd.dma_gather(
                    out_ap=g[:, B * CL :, :],
                    in_ap=v_hi[:],
                    idxs_ap=idxs[:, B * CL * 8 :],
                    num_idxs=B * CH * P,
                    num_idxs_reg=B * CH * P,
                    elem_size=D,
                    single_packet=False,
                )

                for w in range(B):
                    acc = ps.tile([P, D], f32, tag="acc")
                    cols = [w * CL + k for k in range(CL)] + [
                        B * CL + w * CH + k for k in range(CH)
                    ]
                    for k, col in enumerate(cols):
                        S = sp.tile([P, P], f32, tag="S")
                        nc.vector.tensor_scalar(
                            S[:], iota[:], segs[:, col : col + 1], None,
                            mybir.AluOpType.is_equal,
                        )
                        nc.tensor.matmul(
                            acc[:], lhsT=S[:], rhs=g[:, col, :],
                            start=(k == 0), stop=(k == cpw - 1),
                        )
                    o = ob.tile([P, D], f32, tag="o")
                    nc.scalar.activation(
                        o[:], acc[:], mybir.ActivationFunctionType.Copy,
                        scale=rcp[:, w : w + 1],
                    )
                    nc.sync.dma_start(
                        out_t[(b * B + w) * P : (b * B + w + 1) * P, :], o[:]
                    )

    nc.compile()
    return nc


# --------------------------------------------------------------------------
# entry point
# --------------------------------------------------------------------------

def kernel(values, gather_idx, segment_ids, num_segments, _trace=False):
    from concourse.bass_utils import run_bass_kernel_spmd

    num_segments = int(num_segments)
    values = np.asarray(values, dtype=np.float32)
    cfg, in_maps, perm = _build_host(values, gather_idx, segment_ids, num_segments)

    key = (cfg["CL"], cfg["CH"], cfg["NB"], cfg["n_src"], cfg["NWIN"])
    if key not in _COMPILED:
        _COMPILED[key] = _build_program(cfg)
    nc = _COMPILED[key]

    res = run_bass_kernel_spmd(
        nc, in_maps, core_ids=list(range(NCORES)), trace=_trace
    )
    big = np.concatenate([res.results[c]["out"] for c in range(NCORES)], axis=0)
    out = big[perm]
    if _trace:
        kernel.last_exec_time_ns = res.exec_time_ns
        kernel.last_profile = res.profile_json
    return out.astype(np.float32)
